# revision 13
# baseline (speedup 1.0000x reference)
"""ParagraphVector negative-sampling loss kernel for TRN2 (8 NeuronCores).

Sharding: data-parallel over batch. Each of the 8 cores handles 1024 batch
rows; the review/word embedding tables are replicated (only gathered rows are
actually read on-device via indirect DMA).
"""

import numpy as np

import concourse.bass as bass
import concourse.bacc as bacc
import concourse.mybir as mybir
import concourse.tile as tile
from concourse.bass_utils import run_bass_kernel_spmd

B = 8192
W = 8
E = 128
NNEG = 8
K = W * NNEG  # 64
VOCAB = 100000
REVIEW_COUNT = 500000
N_CORES = 8
BC = B // N_CORES  # 1024 rows per core
P = 128
NT = BC // P  # 8 tiles per core

FP32 = mybir.dt.float32
I32 = mybir.dt.int32
ACT = mybir.ActivationFunctionType

_CACHE = {}


def _softplus(nc, pool, out_tag, x_ap, n, neg_input=False):
    """out = softplus(-x) if neg_input else softplus(x), numerically stable.

    softplus(x) = relu(x) + ln(1 + exp(-|x|)); ACT exp/ln tables are only
    accurate for exp args <= 0 (hardware tables diverge for large positive
    args), so keep the exp argument at -|x|.
    """
    negx = pool.tile([P, n], FP32, tag=out_tag + "_n")
    nc.vector.tensor_scalar_mul(out=negx[:], in0=x_ap, scalar1=-1.0)
    a = pool.tile([P, n], FP32, tag=out_tag + "_a")
    nc.vector.tensor_tensor(out=a[:], in0=x_ap, in1=negx[:], op=mybir.AluOpType.min)
    nc.scalar.activation(out=a[:], in_=a[:], func=ACT.Exp)
    nc.scalar.activation(out=a[:], in_=a[:], func=ACT.Ln, bias=1.0)
    r = pool.tile([P, n], FP32, tag=out_tag + "_r")
    nc.vector.tensor_scalar_max(
        out=r[:], in0=negx[:] if neg_input else x_ap, scalar1=0.0
    )
    nc.vector.tensor_tensor(out=a[:], in0=a[:], in1=r[:], op=mybir.AluOpType.add)
    return a


def _indirect_gather_q(nc, out, table, offset_ap, qname):
    """indirect_dma_start(in_offset on axis 0) with an explicit SWDGE queue.

    Mirrors bass.BassGpSimd.indirect_dma_start, which pins queue="qPoolDynamic";
    with num_swdge_queues=4 the extra qPoolDynamic{1..3} rings let descriptor
    generation/service proceed in parallel instead of serializing on one ring.
    """
    g = nc.gpsimd
    out_l = g.lower_ap_dma(out, for_indirect_dma=True)
    in_l = g.lower_ap_dma(table, for_indirect_dma=True)
    off_l = g.lower_ap_dma(offset_ap)
    assert len(out_l) == 1 and len(in_l) == 1 and len(off_l) == 1
    in_l.append(off_l[0])
    shape = table.shape
    coef = 1
    for i in range(1, len(shape)):
        coef *= shape[i]
    in_l[0].dynamic_ap_info = mybir.DynamicAccessPatternInfo(
        c=0,
        actual_ap=out.ap,
        indirect_dim_max_index=shape[0],
        offset_expr=[
            mybir.DynamicAccessPatternOffsetExpr(
                coef=coef,
                aff_expr=mybir.DynamicAccessPatternOffsetExprAffExpr(
                    kind="IndirectArgId", arg_id=1
                ),
            )
        ],
    )
    return g.add_instruction(
        mybir.InstDMACopy(
            name=nc.get_next_instruction_name(),
            queue=qname,
            mode="Copy",
            ins=in_l,
            outs=out_l,
            oob_is_err=True,
            cce_op=mybir.AluOpType.bypass,
        )
    )


N_SWDGE_Q = 4
_QNAMES = ["qPoolDynamic"] + [f"qPoolDynamic{i}" for i in range(1, N_SWDGE_Q)]


def _build_bass():
    nc = bacc.Bacc("TRN2", target_bir_lowering=False, num_swdge_queues=N_SWDGE_Q)

    review_ids = nc.dram_tensor("review_ids", [BC, 1], I32, kind="ExternalInput")
    neg_idx = nc.dram_tensor("neg_idx", [BC, K], I32, kind="ExternalInput")
    rwe = nc.dram_tensor("rwe", [BC, W * E], FP32, kind="ExternalInput")
    mask = nc.dram_tensor("mask", [BC, W], I32, kind="ExternalInput")
    review_table = nc.dram_tensor(
        "review_table", [REVIEW_COUNT, E], FP32, kind="ExternalInput"
    )
    word_table = nc.dram_tensor("word_table", [VOCAB, E], FP32, kind="ExternalInput")

    out_review = nc.dram_tensor("out_review", [BC, E], FP32, kind="ExternalOutput")
    out_loss = nc.dram_tensor("out_loss", [BC, 1], FP32, kind="ExternalOutput")

    with tile.TileContext(nc) as tc:
        with (
            tc.tile_pool(name="big", bufs=3) as big,
            tc.tile_pool(name="small", bufs=3) as small,
            tc.tile_pool(name="persist", bufs=1) as persist,
        ):
            loss_all = persist.tile([P, NT], FP32)

            for t in range(NT):
                rows = bass.ts(t, P)

                idx_tile = small.tile([P, 1], I32, tag="idx")
                negidx_tile = small.tile([P, K], I32, tag="negidx")
                mask_tile = small.tile([P, W], I32, tag="mask")
                rwe_tile = big.tile([P, W * E], FP32, tag="rwe")
                neg_tile = big.tile([P, K * E], FP32, tag="neg")
                review_tile = small.tile([P, E], FP32, tag="review")

                nc.sync.dma_start(out=idx_tile[:], in_=review_ids[rows, :])
                nc.sync.dma_start(out=negidx_tile[:], in_=neg_idx[rows, :])
                nc.sync.dma_start(out=mask_tile[:], in_=mask[rows, :])
                nc.sync.dma_start(out=rwe_tile[:], in_=rwe[rows, :])

                nc.gpsimd.indirect_dma_start(
                    out=review_tile[:],
                    out_offset=None,
                    in_=review_table[:],
                    in_offset=bass.IndirectOffsetOnAxis(ap=idx_tile[:, :1], axis=0),
                )
                for k in range(K):
                    _indirect_gather_q(
                        nc,
                        out=neg_tile[:, k * E : (k + 1) * E],
                        table=word_table[:],
                        offset_ap=negidx_tile[:, k : k + 1],
                        qname=_QNAMES[k % N_SWDGE_Q],
                    )

                nc.sync.dma_start(out=out_review[rows, :], in_=review_tile[:])

                # ---- dots ----
                rev_b_w = review_tile[:].rearrange("p (one e) -> p one e", one=1)
                rev_b_w = rev_b_w.to_broadcast([P, W, E])
                rwe_v = rwe_tile[:].rearrange("p (w e) -> p w e", w=W)
                nc.vector.tensor_tensor(
                    out=rwe_v, in0=rwe_v, in1=rev_b_w, op=mybir.AluOpType.mult
                )
                pos = small.tile([P, W], FP32, tag="pos")
                nc.vector.reduce_sum(out=pos[:], in_=rwe_v, axis=mybir.AxisListType.X)

                rev_b_k = review_tile[:].rearrange("p (one e) -> p one e", one=1)
                rev_b_k = rev_b_k.to_broadcast([P, K, E])
                neg_v = neg_tile[:].rearrange("p (k e) -> p k e", k=K)
                nc.vector.tensor_tensor(
                    out=neg_v, in0=neg_v, in1=rev_b_k, op=mybir.AluOpType.mult
                )
                negdot = small.tile([P, K], FP32, tag="negdot")
                nc.vector.reduce_sum(
                    out=negdot[:], in_=neg_v, axis=mybir.AxisListType.X
                )

                sp_neg = _softplus(nc, small, "sp_neg", negdot[:], K)
                sp_pos = _softplus(nc, small, "sp_pos", pos[:], W, neg_input=True)

                # tok_loss[p, w] = sp_pos[p, w] + sum_j sp_neg[p, w*NNEG + j]
                tok_loss = small.tile([P, W], FP32, tag="tok_loss")
                sp_neg_v = sp_neg[:].rearrange("p (w j) -> p w j", w=W)
                nc.vector.reduce_sum(
                    out=tok_loss[:], in_=sp_neg_v, axis=mybir.AxisListType.X
                )
                nc.vector.tensor_tensor(
                    out=tok_loss[:],
                    in0=tok_loss[:],
                    in1=sp_pos[:],
                    op=mybir.AluOpType.add,
                )

                # ---- masked mean ----
                mask_f = small.tile([P, W], FP32, tag="mask_f")
                nc.vector.tensor_copy(out=mask_f[:], in_=mask_tile[:])

                msum = small.tile([P, 1], FP32, tag="msum")
                nc.vector.reduce_sum(
                    out=msum[:], in_=mask_f[:], axis=mybir.AxisListType.X
                )
                nc.vector.tensor_scalar_max(out=msum[:], in0=msum[:], scalar1=1.0)
                rinv = small.tile([P, 1], FP32, tag="rinv")
                nc.vector.reciprocal(out=rinv[:], in_=msum[:])

                loss_num = small.tile([P, 1], FP32, tag="loss_num")
                nc.vector.tensor_tensor(
                    out=tok_loss[:],
                    in0=tok_loss[:],
                    in1=mask_f[:],
                    op=mybir.AluOpType.mult,
                )
                nc.vector.reduce_sum(
                    out=loss_num[:], in_=tok_loss[:], axis=mybir.AxisListType.X
                )
                nc.vector.tensor_tensor(
                    out=loss_all[:, t : t + 1],
                    in0=loss_num[:],
                    in1=rinv[:],
                    op=mybir.AluOpType.mult,
                )

            out_loss_v = out_loss.rearrange("(t p) one -> p (t one)", p=P)
            nc.sync.dma_start(out=out_loss_v, in_=loss_all[:])

    nc.compile()
    return nc


def kernel(review_ids, review_word_emb, review_word_mask, neg_sample_idxs,
           n_negs, review_emb_table, word_emb_table):
    review_ids = np.ascontiguousarray(
        np.asarray(review_ids, dtype=np.int32).reshape(B, 1)
    )
    neg_idx = np.ascontiguousarray(
        np.asarray(neg_sample_idxs, dtype=np.int32).reshape(B, K)
    )
    rwe = np.ascontiguousarray(
        np.asarray(review_word_emb, dtype=np.float32).reshape(B, W * E)
    )
    mask = np.ascontiguousarray(
        np.asarray(review_word_mask, dtype=np.int32).reshape(B, W)
    )
    review_table = np.ascontiguousarray(
        np.asarray(review_emb_table, dtype=np.float32).reshape(REVIEW_COUNT, E)
    )
    word_table = np.ascontiguousarray(
        np.asarray(word_emb_table, dtype=np.float32).reshape(VOCAB, E)
    )

    if "nc" not in _CACHE:
        _CACHE["nc"] = _build_bass()
    nc = _CACHE["nc"]

    in_maps = []
    for c in range(N_CORES):
        s = slice(c * BC, (c + 1) * BC)
        in_maps.append(
            {
                "review_ids": review_ids[s],
                "neg_idx": neg_idx[s],
                "rwe": rwe[s],
                "mask": mask[s],
                "review_table": review_table,
                "word_table": word_table,
            }
        )

    res = run_bass_kernel_spmd(nc, in_maps, list(range(N_CORES)))
    _CACHE["last_results"] = res

    review_emb = np.concatenate([r["out_review"] for r in res.results], axis=0)
    loss = np.concatenate([r["out_loss"] for r in res.results], axis=0)
    return review_emb, loss


# revision 15
# speedup vs baseline: 1.0318x; 1.0318x over previous
"""ParagraphVector negative-sampling loss kernel for TRN2 (8 NeuronCores).

Sharding: data-parallel over batch. Each of the 8 cores handles 1024 batch
rows; the review/word embedding tables are replicated (only gathered rows are
actually read on-device via indirect DMA).
"""

import numpy as np

import concourse.bass as bass
import concourse.bacc as bacc
import concourse.mybir as mybir
import concourse.tile as tile
from concourse.bass_utils import run_bass_kernel_spmd

B = 8192
W = 8
E = 128
NNEG = 8
K = W * NNEG  # 64
VOCAB = 100000
REVIEW_COUNT = 500000
N_CORES = 8
BC = B // N_CORES  # 1024 rows per core
P = 128
NT = BC // P  # 8 tiles per core

FP32 = mybir.dt.float32
I32 = mybir.dt.int32
ACT = mybir.ActivationFunctionType

_CACHE = {}


def _softplus(nc, pool, out_tag, x_ap, n, neg_input=False):
    """out = softplus(-x) if neg_input else softplus(x), numerically stable.

    softplus(x) = relu(x) + ln(1 + exp(-|x|)); ACT exp/ln tables are only
    accurate for exp args <= 0 (hardware tables diverge for large positive
    args), so keep the exp argument at -|x|.
    """
    negx = pool.tile([P, n], FP32, tag=out_tag + "_n")
    nc.vector.tensor_scalar_mul(out=negx[:], in0=x_ap, scalar1=-1.0)
    a = pool.tile([P, n], FP32, tag=out_tag + "_a")
    nc.vector.tensor_tensor(out=a[:], in0=x_ap, in1=negx[:], op=mybir.AluOpType.min)
    nc.scalar.activation(out=a[:], in_=a[:], func=ACT.Exp)
    nc.scalar.activation(out=a[:], in_=a[:], func=ACT.Ln, bias=1.0)
    r = pool.tile([P, n], FP32, tag=out_tag + "_r")
    nc.vector.tensor_scalar_max(
        out=r[:], in0=negx[:] if neg_input else x_ap, scalar1=0.0
    )
    nc.vector.tensor_tensor(out=a[:], in0=a[:], in1=r[:], op=mybir.AluOpType.add)
    return a


def _build_bass():
    nc = bacc.Bacc("TRN2", target_bir_lowering=False)

    review_ids = nc.dram_tensor("review_ids", [BC, 1], I32, kind="ExternalInput")
    neg_idx = nc.dram_tensor("neg_idx", [BC, K], I32, kind="ExternalInput")
    rwe = nc.dram_tensor("rwe", [BC, W * E], FP32, kind="ExternalInput")
    mask = nc.dram_tensor("mask", [BC, W], I32, kind="ExternalInput")
    review_table = nc.dram_tensor(
        "review_table", [REVIEW_COUNT, E], FP32, kind="ExternalInput"
    )
    word_table = nc.dram_tensor("word_table", [VOCAB, E], FP32, kind="ExternalInput")

    out_review = nc.dram_tensor("out_review", [BC, E], FP32, kind="ExternalOutput")
    out_loss = nc.dram_tensor("out_loss", [BC, 1], FP32, kind="ExternalOutput")

    with tile.TileContext(nc) as tc:
        with (
            tc.tile_pool(name="big", bufs=3) as big,
            tc.tile_pool(name="small", bufs=3) as small,
            tc.tile_pool(name="persist", bufs=1) as persist,
        ):
            loss_all = persist.tile([P, NT], FP32)

            for t in range(NT):
                rows = bass.ts(t, P)

                idx_tile = small.tile([P, 1], I32, tag="idx")
                negidx_tile = small.tile([P, K], I32, tag="negidx")
                mask_tile = small.tile([P, W], I32, tag="mask")
                rwe_tile = big.tile([P, W * E], FP32, tag="rwe")
                neg_tile = big.tile([P, K * E], FP32, tag="neg")
                review_tile = small.tile([P, E], FP32, tag="review")

                nc.sync.dma_start(out=idx_tile[:], in_=review_ids[rows, :])
                nc.sync.dma_start(out=negidx_tile[:], in_=neg_idx[rows, :])
                nc.sync.dma_start(out=mask_tile[:], in_=mask[rows, :])
                nc.sync.dma_start(out=rwe_tile[:], in_=rwe[rows, :])

                nc.gpsimd.indirect_dma_start(
                    out=review_tile[:],
                    out_offset=None,
                    in_=review_table[:],
                    in_offset=bass.IndirectOffsetOnAxis(ap=idx_tile[:, :1], axis=0),
                )
                # ---- pos dot (independent of neg gathers) ----
                rev_b_w = review_tile[:].rearrange("p (one e) -> p one e", one=1)
                rev_b_w = rev_b_w.to_broadcast([P, W, E])
                rwe_v = rwe_tile[:].rearrange("p (w e) -> p w e", w=W)
                nc.vector.tensor_tensor(
                    out=rwe_v, in0=rwe_v, in1=rev_b_w, op=mybir.AluOpType.mult
                )
                pos = small.tile([P, W], FP32, tag="pos")
                nc.vector.reduce_sum(out=pos[:], in_=rwe_v, axis=mybir.AxisListType.X)

                nc.sync.dma_start(out=out_review[rows, :], in_=review_tile[:])

                # ---- neg gathers chunked with partial dots so the final
                # tile's vector work mostly overlaps the gather stream ----
                KC = 16
                negdot = small.tile([P, K], FP32, tag="negdot")
                rev_b_c = review_tile[:].rearrange("p (one e) -> p one e", one=1)
                rev_b_c = rev_b_c.to_broadcast([P, KC, E])
                for c in range(K // KC):
                    for k in range(c * KC, (c + 1) * KC):
                        nc.gpsimd.indirect_dma_start(
                            out=neg_tile[:, k * E : (k + 1) * E],
                            out_offset=None,
                            in_=word_table[:],
                            in_offset=bass.IndirectOffsetOnAxis(
                                ap=negidx_tile[:, k : k + 1], axis=0
                            ),
                        )
                    seg = neg_tile[:, c * KC * E : (c + 1) * KC * E]
                    seg_v = seg.rearrange("p (k e) -> p k e", k=KC)
                    nc.vector.tensor_tensor(
                        out=seg_v, in0=seg_v, in1=rev_b_c, op=mybir.AluOpType.mult
                    )
                    nc.vector.reduce_sum(
                        out=negdot[:, c * KC : (c + 1) * KC],
                        in_=seg_v,
                        axis=mybir.AxisListType.X,
                    )

                sp_neg = _softplus(nc, small, "sp_neg", negdot[:], K)
                sp_pos = _softplus(nc, small, "sp_pos", pos[:], W, neg_input=True)

                # tok_loss[p, w] = sp_pos[p, w] + sum_j sp_neg[p, w*NNEG + j]
                tok_loss = small.tile([P, W], FP32, tag="tok_loss")
                sp_neg_v = sp_neg[:].rearrange("p (w j) -> p w j", w=W)
                nc.vector.reduce_sum(
                    out=tok_loss[:], in_=sp_neg_v, axis=mybir.AxisListType.X
                )
                nc.vector.tensor_tensor(
                    out=tok_loss[:],
                    in0=tok_loss[:],
                    in1=sp_pos[:],
                    op=mybir.AluOpType.add,
                )

                # ---- masked mean ----
                mask_f = small.tile([P, W], FP32, tag="mask_f")
                nc.vector.tensor_copy(out=mask_f[:], in_=mask_tile[:])

                msum = small.tile([P, 1], FP32, tag="msum")
                nc.vector.reduce_sum(
                    out=msum[:], in_=mask_f[:], axis=mybir.AxisListType.X
                )
                nc.vector.tensor_scalar_max(out=msum[:], in0=msum[:], scalar1=1.0)
                rinv = small.tile([P, 1], FP32, tag="rinv")
                nc.vector.reciprocal(out=rinv[:], in_=msum[:])

                loss_num = small.tile([P, 1], FP32, tag="loss_num")
                nc.vector.tensor_tensor(
                    out=tok_loss[:],
                    in0=tok_loss[:],
                    in1=mask_f[:],
                    op=mybir.AluOpType.mult,
                )
                nc.vector.reduce_sum(
                    out=loss_num[:], in_=tok_loss[:], axis=mybir.AxisListType.X
                )
                nc.vector.tensor_tensor(
                    out=loss_all[:, t : t + 1],
                    in0=loss_num[:],
                    in1=rinv[:],
                    op=mybir.AluOpType.mult,
                )

            out_loss_v = out_loss.rearrange("(t p) one -> p (t one)", p=P)
            nc.sync.dma_start(out=out_loss_v, in_=loss_all[:])

    nc.compile()
    return nc


def kernel(review_ids, review_word_emb, review_word_mask, neg_sample_idxs,
           n_negs, review_emb_table, word_emb_table):
    review_ids = np.ascontiguousarray(
        np.asarray(review_ids, dtype=np.int32).reshape(B, 1)
    )
    neg_idx = np.ascontiguousarray(
        np.asarray(neg_sample_idxs, dtype=np.int32).reshape(B, K)
    )
    rwe = np.ascontiguousarray(
        np.asarray(review_word_emb, dtype=np.float32).reshape(B, W * E)
    )
    mask = np.ascontiguousarray(
        np.asarray(review_word_mask, dtype=np.int32).reshape(B, W)
    )
    review_table = np.ascontiguousarray(
        np.asarray(review_emb_table, dtype=np.float32).reshape(REVIEW_COUNT, E)
    )
    word_table = np.ascontiguousarray(
        np.asarray(word_emb_table, dtype=np.float32).reshape(VOCAB, E)
    )

    if "nc" not in _CACHE:
        _CACHE["nc"] = _build_bass()
    nc = _CACHE["nc"]

    in_maps = []
    for c in range(N_CORES):
        s = slice(c * BC, (c + 1) * BC)
        in_maps.append(
            {
                "review_ids": review_ids[s],
                "neg_idx": neg_idx[s],
                "rwe": rwe[s],
                "mask": mask[s],
                "review_table": review_table,
                "word_table": word_table,
            }
        )

    res = run_bass_kernel_spmd(nc, in_maps, list(range(N_CORES)))
    _CACHE["last_results"] = res

    review_emb = np.concatenate([r["out_review"] for r in res.results], axis=0)
    loss = np.concatenate([r["out_loss"] for r in res.results], axis=0)
    return review_emb, loss
